# revision 1
# baseline (speedup 1.0000x reference)
"""Conv1D-RNN Trainium2 kernel.

h_t = tanh(conv1d(x_t, W_i) + conv1d(h_{t-1}, W_h)), K=3, pad=1.
x: [B=16, T=64, C=128, L=128] -> out: [B=16, T=64, H=256, L=128], fp32.

Sharding: data-parallel over batch, 2 batches per core across 8 cores.
Weights replicated. The T-recurrence runs on-core; each conv is expressed
as 3 shifted matmuls (one per tap) accumulating into PSUM over a
zero-padded [*, L+2] SBUF layout, with the hidden dim H=256 split into
two 128-row chunks. All matmuls run in float32r (full PE rate at
free-dim 256, ~1e-4 relative precision). tanh is applied straight out
of PSUM by the scalar engine, writing the f32r hidden-state ring that
feeds the next step's matmuls.
"""

import numpy as np

B, T, C, H, L, K = 16, 64, 128, 256, 128, 3
NCORES = 8
BL = B // NCORES          # batches per core
NHC = H // 128            # hidden chunks of 128
LP = L + 2                # padded length (zero col at each end)
XB = 6                    # x ring depth
PF = 3                    # x DMA prefetch distance (steps)
RING = 4                  # h ring depth

_CACHE = {}


def build(repeat=1):
    """Build + compile the per-core Bass program. repeat>1 wraps the whole
    T-loop in a hardware For-loop (used only for timing runs)."""
    import concourse.bacc as bacc
    import concourse.mybir as mybir
    import concourse.tile as tile

    dt = mybir.dt
    f32 = dt.float32
    f32r = dt.float32r

    nc = bacc.Bacc("TRN2", target_bir_lowering=False, debug=False)
    # x arrives host-padded to LP columns (zeros at 0 and L+1), so the DMA
    # fills entire ring slots and no on-device pad zeroing is needed
    x_d = nc.dram_tensor("x", [BL, T, C, LP], f32r, kind="ExternalInput")
    wi_d = nc.dram_tensor("wi", [K, NHC, C, 128], f32r, kind="ExternalInput")
    wh_d = nc.dram_tensor("wh", [K, NHC, NHC, 128, 128], f32r, kind="ExternalInput")
    o_d = nc.dram_tensor("o", [BL, T, H, L], f32, kind="ExternalOutput")

    with tile.TileContext(nc) as tc, tc.tile_pool(name="persist", bufs=1) as persist:
        wi_s = persist.tile([128, K, NHC, 128], f32r, name="wi_s")
        wh_s = persist.tile([128, K, NHC, NHC, 128], f32r, name="wh_s")
        xr = persist.tile([128, XB, BL, LP], f32r, name="xr")
        hr0 = persist.tile([128, RING, BL, LP], f32r, name="hr0")
        hr1 = persist.tile([128, RING, BL, LP], f32r, name="hr1")
        hr = [hr0, hr1]
        warm = persist.tile([128, 256], f32r, name="warm")

        # PE warmup: the PE otherwise idles ~4us waiting for the first loads
        # at the cold 1.2GHz HAM clock; dummy matmuls during that window trip
        # the activity monitor to 2.4GHz before the first real matmul
        nc.gpsimd.memset(warm[:].bitcast(f32), 0.0)
        with tc.tile_pool(name="warm_ps", bufs=1, space="PSUM") as wpool:
            wps = wpool.tile([128, 256], f32, name="wps")
            for _ in range(20):
                nc.tensor.matmul(
                    wps[:], lhsT=warm[:, :128], rhs=warm[:], start=True, stop=True
                )

        def xdma(t):
            # issued from the Activation engine: its own HWDGE queue, so x
            # prefetches never sit behind the output-store queue
            nc.scalar.dma_start(
                out=xr[:, t % XB],
                in_=x_d.ap()[:, t].rearrange("b c l -> c b l"),
            )

        # startup loads: x(0..PF-1) first on the ACT queue, wi in parallel on
        # the SP queue, then the big wh behind the x prefetches on ACT
        for t in range(PF):
            xdma(t)
        nc.sync.dma_start(out=wi_s[:], in_=wi_d.ap().rearrange("k hc c h -> c k hc h"))
        for k in range(K):
            nc.sync.dma_start(
                out=wh_s[:, k],
                in_=wh_d.ap()[k].rearrange("hc cc c h -> c hc cc h"),
            )

        # NOTE: in repeat>1 (timing) mode, iterations >=1 reread the x ring
        # slots for t<PF with stale data — identical work, values differ.
        def body():
            for hc in range(NHC):
                # h_{-1} = 0: zero the slot step 0 reads, plus all pad cols
                nc.gpsimd.memset(hr[hc][:, RING - 1].bitcast(f32), 0.0)
                nc.gpsimd.memset(hr[hc][:, :, :, :: LP - 1].bitcast(f32), 0.0)
            with tc.tile_pool(name="ps", bufs=4, space="PSUM") as pspool:
                for t in range(T):
                    xs = t % XB
                    hs = t % RING
                    hp = (t - 1) % RING
                    ps = [
                        pspool.tile([128, BL, L], f32, name=f"ps{t}_{hc}", tag="ps")
                        for hc in range(NHC)
                    ]
                    # input conv first: no dependence on h, keeps PE busy
                    # while the previous step's tanh completes
                    for hc in range(NHC):
                        for k in range(K):
                            nc.tensor.matmul(
                                ps[hc][:],
                                lhsT=wi_s[:, k, hc, :],
                                rhs=xr[:, xs, :, k : k + L],
                                start=(k == 0),
                                stop=False,
                            )
                    # cc-major order: everything depending on h chunk 0 runs
                    # before anything depending on h chunk 1, maximizing the
                    # window for the second tanh of the previous step
                    for cc in range(NHC):
                        for hc in range(NHC):
                            for k in range(K):
                                nc.tensor.matmul(
                                    ps[hc][:],
                                    lhsT=wh_s[:, k, hc, cc, :],
                                    rhs=hr[cc][:, hp, :, k : k + L],
                                    start=False,
                                    stop=(cc == NHC - 1 and k == K - 1),
                                )
                    for hc in range(NHC):
                        nc.scalar.activation(
                            hr[hc][:, hs, :, 1 : L + 1],
                            ps[hc][:],
                            mybir.ActivationFunctionType.Tanh,
                        )
                    # x prefetch issued after the tanhs so its ACT-sequencer
                    # slot never delays tanh dispatch (still PF steps ahead)
                    if t + PF < T:
                        xdma(t + PF)
                    for hc in range(NHC):
                        # stores on the SP queue; at the final step the ACT
                        # queue is idle, so parallelize the two last stores
                        eng = nc.scalar if (t == T - 1 and hc == 1) else nc.sync
                        eng.dma_start(
                            out=o_d.ap()[:, t, hc * 128 : (hc + 1) * 128, :].rearrange(
                                "b h l -> h b l"
                            ),
                            in_=hr[hc][:, hs, :, 1 : L + 1].bitcast(f32),
                        )

        if repeat == 1:
            body()
        else:
            with tc.For_i(0, repeat, 1):
                body()

    nc.compile()
    return nc


def _get(repeat=1):
    if repeat not in _CACHE:
        _CACHE[repeat] = build(repeat)
    return _CACHE[repeat]


def prep_weights(W_i, W_h):
    wi = np.ascontiguousarray(
        W_i.transpose(2, 1, 0).reshape(K, C, NHC, 128).transpose(0, 2, 1, 3)
    )
    wh = np.ascontiguousarray(
        W_h.transpose(2, 1, 0).reshape(K, NHC, 128, NHC, 128).transpose(0, 3, 1, 2, 4)
    )
    return wi, wh


def make_in_maps(x, W_i, W_h):
    wi, wh = prep_weights(np.asarray(W_i, np.float32), np.asarray(W_h, np.float32))
    x = np.asarray(x, np.float32)
    xp = np.zeros((B, T, C, LP), np.float32)
    xp[:, :, :, 1 : L + 1] = x
    return [
        {"x": np.ascontiguousarray(xp[BL * c : BL * (c + 1)]), "wi": wi, "wh": wh}
        for c in range(NCORES)
    ]


def kernel(x, W_i, W_h):
    from concourse.bass_utils import run_bass_kernel_spmd

    nc = _get(1)
    in_maps = make_in_maps(x, W_i, W_h)
    res = run_bass_kernel_spmd(nc, in_maps, core_ids=list(range(NCORES)))
    out = np.concatenate([res.results[c]["o"] for c in range(NCORES)], axis=0)
    return np.ascontiguousarray(out.astype(np.float32))



# revision 2
# speedup vs baseline: 1.2707x; 1.2707x over previous
"""Conv1D-RNN Trainium2 kernel.

h_t = tanh(conv1d(x_t, W_i) + conv1d(h_{t-1}, W_h)), K=3, pad=1.
x: [B=16, T=64, C=128, L=128] -> out: [B=16, T=64, H=256, L=128], fp32.

Sharding: data-parallel over batch, 2 batches per core across 8 cores.
Weights replicated. The T-recurrence runs on-core; each conv is expressed
as 3 shifted matmuls (one per tap) accumulating into PSUM over a
zero-padded [*, L+2] SBUF layout, with the hidden dim H=256 split into
two 128-row chunks. All operands are fp16 (same 1 cycle/row PE rate as
f32r at this size, but half the SBUF/DMA traffic); accumulation stays
fp32 in PSUM. tanh is applied straight out of PSUM by the scalar engine,
writing the fp16 hidden-state ring that feeds the next step's matmuls.
The output rides to DRAM as fp16 and is upcast to fp32 on the host
(adds only one final rounding, ~1e-4, on top of the recurrent fp16
path; measured rel err ~4e-3 vs the 2e-2 gate).
"""

import numpy as np

B, T, C, H, L, K = 16, 64, 128, 256, 128, 3
NCORES = 8
BL = B // NCORES          # batches per core
NHC = H // 128            # hidden chunks of 128
LP = L + 2                # padded length (zero col at each end)
XB = 6                    # x ring depth
PF = 3                    # x DMA prefetch distance (steps)
RING = 4                  # h ring depth

_CACHE = {}


def build(repeat=1):
    """Build + compile the per-core Bass program. repeat>1 wraps the whole
    T-loop in a hardware For-loop (used only for timing runs)."""
    import concourse.bacc as bacc
    import concourse.mybir as mybir
    import concourse.tile as tile

    dt = mybir.dt
    f16 = dt.float16
    f32 = dt.float32

    nc = bacc.Bacc("TRN2", target_bir_lowering=False, debug=False)
    # x arrives host-padded to LP columns (zeros at 0 and L+1), so the DMA
    # fills entire ring slots and no on-device pad zeroing is needed
    x_d = nc.dram_tensor("x", [BL, T, C, LP], f16, kind="ExternalInput")
    wi_d = nc.dram_tensor("wi", [K, NHC, C, 128], f16, kind="ExternalInput")
    wh_d = nc.dram_tensor("wh", [K, NHC, NHC, 128, 128], f16, kind="ExternalInput")
    o_d = nc.dram_tensor("o", [BL, T, H, L], f16, kind="ExternalOutput")

    with tile.TileContext(nc) as tc, tc.tile_pool(name="persist", bufs=1) as persist:
        wi_s = persist.tile([128, K, NHC, 128], f16, name="wi_s")
        wh_s = persist.tile([128, K, NHC, NHC, 128], f16, name="wh_s")
        xr = persist.tile([128, XB, BL, LP], f16, name="xr")
        hr0 = persist.tile([128, RING, BL, LP], f16, name="hr0")
        hr1 = persist.tile([128, RING, BL, LP], f16, name="hr1")
        hr = [hr0, hr1]
        warm = persist.tile([128, 256], f16, name="warm")

        # PE warmup: the PE otherwise idles ~4us waiting for the first loads
        # at the cold 1.2GHz HAM clock; dummy matmuls during that window trip
        # the activity monitor to 2.4GHz before the first real matmul
        nc.gpsimd.memset(warm[:], 0.0)
        with tc.tile_pool(name="warm_ps", bufs=1, space="PSUM") as wpool:
            wps = wpool.tile([128, 256], f32, name="wps")
            for _ in range(20):
                nc.tensor.matmul(
                    wps[:], lhsT=warm[:, :128], rhs=warm[:], start=True, stop=True
                )

        def xdma(t):
            # issued from the Activation engine: its own HWDGE queue, so x
            # prefetches never sit behind the output-store queue
            nc.scalar.dma_start(
                out=xr[:, t % XB],
                in_=x_d.ap()[:, t].rearrange("b c l -> c b l"),
            )

        # startup loads: x(0..PF-1) first on the ACT queue, wi in parallel on
        # the SP queue, then the big wh behind the x prefetches on ACT
        for t in range(PF):
            xdma(t)
        nc.sync.dma_start(out=wi_s[:], in_=wi_d.ap().rearrange("k hc c h -> c k hc h"))
        for k in range(K):
            nc.sync.dma_start(
                out=wh_s[:, k],
                in_=wh_d.ap()[k].rearrange("hc cc c h -> c hc cc h"),
            )

        # NOTE: in repeat>1 (timing) mode, iterations >=1 reread the x ring
        # slots for t<PF with stale data — identical work, values differ.
        def body():
            for hc in range(NHC):
                # h_{-1} = 0: zero the slot step 0 reads, plus all pad cols
                nc.gpsimd.memset(hr[hc][:, RING - 1], 0.0)
                nc.gpsimd.memset(hr[hc][:, :, :, :: LP - 1], 0.0)
            with tc.tile_pool(name="ps", bufs=4, space="PSUM") as pspool:
                for t in range(T):
                    xs = t % XB
                    hs = t % RING
                    hp = (t - 1) % RING
                    ps = [
                        pspool.tile([128, BL, L], f32, name=f"ps{t}_{hc}", tag="ps")
                        for hc in range(NHC)
                    ]
                    # input conv first: no dependence on h, keeps PE busy
                    # while the previous step's tanh completes
                    for hc in range(NHC):
                        for k in range(K):
                            nc.tensor.matmul(
                                ps[hc][:],
                                lhsT=wi_s[:, k, hc, :],
                                rhs=xr[:, xs, :, k : k + L],
                                start=(k == 0),
                                stop=False,
                            )
                    # cc-major order: everything depending on h chunk 0 runs
                    # before anything depending on h chunk 1, maximizing the
                    # window for the second tanh of the previous step
                    for cc in range(NHC):
                        for hc in range(NHC):
                            for k in range(K):
                                nc.tensor.matmul(
                                    ps[hc][:],
                                    lhsT=wh_s[:, k, hc, cc, :],
                                    rhs=hr[cc][:, hp, :, k : k + L],
                                    start=False,
                                    stop=(cc == NHC - 1 and k == K - 1),
                                )
                    for hc in range(NHC):
                        nc.scalar.activation(
                            hr[hc][:, hs, :, 1 : L + 1],
                            ps[hc][:],
                            mybir.ActivationFunctionType.Tanh,
                        )
                    # x prefetch issued after the tanhs so its ACT-sequencer
                    # slot never delays tanh dispatch (still PF steps ahead)
                    if t + PF < T:
                        xdma(t + PF)
                    for hc in range(NHC):
                        # stores on the SP queue; at the final step the ACT
                        # queue is idle, so parallelize the two last stores
                        eng = nc.scalar if (t == T - 1 and hc == 1) else nc.sync
                        eng.dma_start(
                            out=o_d.ap()[:, t, hc * 128 : (hc + 1) * 128, :].rearrange(
                                "b h l -> h b l"
                            ),
                            in_=hr[hc][:, hs, :, 1 : L + 1],
                        )

        if repeat == 1:
            body()
        else:
            with tc.For_i(0, repeat, 1):
                body()

    nc.compile()
    return nc


def _get(repeat=1):
    if repeat not in _CACHE:
        _CACHE[repeat] = build(repeat)
    return _CACHE[repeat]


def prep_weights(W_i, W_h):
    wi = np.ascontiguousarray(
        W_i.transpose(2, 1, 0).reshape(K, C, NHC, 128).transpose(0, 2, 1, 3)
    ).astype(np.float16)
    wh = np.ascontiguousarray(
        W_h.transpose(2, 1, 0).reshape(K, NHC, 128, NHC, 128).transpose(0, 3, 1, 2, 4)
    ).astype(np.float16)
    return wi, wh


def make_in_maps(x, W_i, W_h):
    wi, wh = prep_weights(np.asarray(W_i, np.float32), np.asarray(W_h, np.float32))
    x = np.asarray(x, np.float32)
    xp = np.zeros((B, T, C, LP), np.float16)
    xp[:, :, :, 1 : L + 1] = x
    return [
        {"x": np.ascontiguousarray(xp[BL * c : BL * (c + 1)]), "wi": wi, "wh": wh}
        for c in range(NCORES)
    ]


def kernel(x, W_i, W_h):
    from concourse.bass_utils import run_bass_kernel_spmd

    nc = _get(1)
    in_maps = make_in_maps(x, W_i, W_h)
    res = run_bass_kernel_spmd(nc, in_maps, core_ids=list(range(NCORES)))
    out = np.concatenate([res.results[c]["o"] for c in range(NCORES)], axis=0)
    return np.ascontiguousarray(out.astype(np.float32))


# revision 3
# speedup vs baseline: 1.2945x; 1.0187x over previous
"""Conv1D-RNN Trainium2 kernel.

h_t = tanh(conv1d(x_t, W_i) + conv1d(h_{t-1}, W_h)), K=3, pad=1.
x: [B=16, T=64, C=128, L=128] -> out: [B=16, T=64, H=256, L=128], fp32.

Sharding: data-parallel over batch, 2 batches per core across 8 cores.
Weights replicated. The T-recurrence runs on-core; each conv is expressed
as 3 shifted matmuls (one per tap) accumulating into PSUM over a
zero-padded [L+2, BL] SBUF layout (position-major, batch innermost: every
DMA line is then >=512B contiguous on both sides, which keeps the DMA
engines at full descriptor rate), with the hidden dim H=256 split into
two 128-row chunks. All operands are fp16 (measurably faster per matmul
row than f32r on TRN2 and half the SBUF/DMA traffic); accumulation stays
fp32 in PSUM. tanh is applied straight out of PSUM by the scalar engine,
writing the fp16 hidden-state ring that feeds the next step's matmuls.
The output rides to DRAM as fp16 [T, H, L, BL] and is transposed/upcast
on the host (adds only one final rounding ~1e-4 on top of the recurrent
fp16 path; measured rel err ~4e-3 vs the 2e-2 gate).
"""

import numpy as np

B, T, C, H, L, K = 16, 64, 128, 256, 128, 3
NCORES = 8
BL = B // NCORES          # batches per core
NHC = H // 128            # hidden chunks of 128
LP = L + 2                # padded length (zero col at each end)
XB = 6                    # x ring depth
PF = 3                    # x DMA prefetch distance (steps)
RING = 4                  # h ring depth
PSB = 8                   # PSUM pool depth (4 steps in flight)

_CACHE = {}


def build(repeat=1):
    """Build + compile the per-core Bass program. repeat>1 wraps the whole
    T-loop in a hardware For-loop (used only for timing runs)."""
    import concourse.bacc as bacc
    import concourse.mybir as mybir
    import concourse.tile as tile

    dt = mybir.dt
    f16 = dt.float16
    f32 = dt.float32

    nc = bacc.Bacc("TRN2", target_bir_lowering=False, debug=False)
    # x arrives host-padded to LP rows (zeros at 0 and L+1), batch innermost
    x_d = nc.dram_tensor("x", [T, C, LP, BL], f16, kind="ExternalInput")
    wi_d = nc.dram_tensor("wi", [K, NHC, C, 128], f16, kind="ExternalInput")
    wh_d = nc.dram_tensor("wh", [K, NHC, NHC, 128, 128], f16, kind="ExternalInput")
    o_d = nc.dram_tensor("o", [T, H, L, BL], f16, kind="ExternalOutput")

    with tile.TileContext(nc) as tc, tc.tile_pool(name="persist", bufs=1) as persist:
        wi_s = persist.tile([128, K, NHC, 128], f16, name="wi_s")
        wh_s = persist.tile([128, K, NHC, NHC, 128], f16, name="wh_s")
        xr = persist.tile([128, XB, LP, BL], f16, name="xr")
        hr0 = persist.tile([128, RING, LP, BL], f16, name="hr0")
        hr1 = persist.tile([128, RING, LP, BL], f16, name="hr1")
        hr = [hr0, hr1]
        warm = persist.tile([128, 256], f16, name="warm")

        # PE warmup: the PE otherwise idles ~4us waiting for the first loads
        # at the cold 1.2GHz HAM clock; dummy matmuls during that window trip
        # the activity monitor to 2.4GHz before the first real matmul
        nc.gpsimd.memset(warm[:], 0.0)
        with tc.tile_pool(name="warm_ps", bufs=1, space="PSUM") as wpool:
            wps = wpool.tile([128, 256], f32, name="wps")
            for _ in range(20):
                nc.tensor.matmul(
                    wps[:], lhsT=warm[:, :128], rhs=warm[:], start=True, stop=True
                )

        def xdma(t):
            # issued from the Activation engine: its own HWDGE queue, so x
            # prefetches never sit behind the output-store queue
            nc.scalar.dma_start(out=xr[:, t % XB], in_=x_d.ap()[t])

        # startup loads: x(0..PF-1) first on the ACT queue, wi in parallel on
        # the SP queue, then the big wh behind the x prefetches on ACT
        for t in range(PF):
            xdma(t)
        nc.sync.dma_start(out=wi_s[:], in_=wi_d.ap().rearrange("k hc c h -> c k hc h"))
        for k in range(K):
            nc.sync.dma_start(
                out=wh_s[:, k],
                in_=wh_d.ap()[k].rearrange("hc cc c h -> c hc cc h"),
            )

        # NOTE: in repeat>1 (timing) mode, iterations >=1 reread the x ring
        # slots for t<PF with stale data — identical work, values differ.
        def body():
            for hc in range(NHC):
                # h_{-1} = 0: zero the slot step 0 reads, plus all pad rows
                nc.gpsimd.memset(hr[hc][:, RING - 1], 0.0)
                nc.gpsimd.memset(hr[hc][:, :, :: LP - 1], 0.0)
            with tc.tile_pool(name="ps", bufs=PSB, space="PSUM") as pspool:
                for t in range(T):
                    xs = t % XB
                    hs = t % RING
                    hp = (t - 1) % RING
                    ps = [
                        pspool.tile([128, L, BL], f32, name=f"ps{t}_{hc}", tag="ps")
                        for hc in range(NHC)
                    ]
                    # input conv first: no dependence on h, keeps PE busy
                    # while the previous step's tanh completes
                    for hc in range(NHC):
                        for k in range(K):
                            nc.tensor.matmul(
                                ps[hc][:],
                                lhsT=wi_s[:, k, hc, :],
                                rhs=xr[:, xs, k : k + L],
                                start=(k == 0),
                                stop=False,
                            )
                    # cc-major order: everything depending on h chunk 0 runs
                    # before anything depending on h chunk 1, maximizing the
                    # window for the second tanh of the previous step
                    for cc in range(NHC):
                        for hc in range(NHC):
                            for k in range(K):
                                nc.tensor.matmul(
                                    ps[hc][:],
                                    lhsT=wh_s[:, k, hc, cc, :],
                                    rhs=hr[cc][:, hp, k : k + L],
                                    start=False,
                                    stop=(cc == NHC - 1 and k == K - 1),
                                )
                    for hc in range(NHC):
                        nc.scalar.activation(
                            hr[hc][:, hs, 1 : L + 1],
                            ps[hc][:],
                            mybir.ActivationFunctionType.Tanh,
                        )
                    # x prefetch issued after the tanhs so its ACT-sequencer
                    # slot never delays tanh dispatch (still PF steps ahead)
                    if t + PF < T:
                        xdma(t + PF)
                    for hc in range(NHC):
                        # stores on the SP queue; at the final step the ACT
                        # queue is idle, so parallelize the two last stores
                        eng = nc.scalar if (t == T - 1 and hc == 1) else nc.sync
                        eng.dma_start(
                            out=o_d.ap()[t, hc * 128 : (hc + 1) * 128],
                            in_=hr[hc][:, hs, 1 : L + 1],
                        )

        if repeat == 1:
            body()
        else:
            with tc.For_i(0, repeat, 1):
                body()

    nc.compile()
    return nc


def _get(repeat=1):
    if repeat not in _CACHE:
        _CACHE[repeat] = build(repeat)
    return _CACHE[repeat]


def prep_weights(W_i, W_h):
    wi = np.ascontiguousarray(
        W_i.transpose(2, 1, 0).reshape(K, C, NHC, 128).transpose(0, 2, 1, 3)
    ).astype(np.float16)
    wh = np.ascontiguousarray(
        W_h.transpose(2, 1, 0).reshape(K, NHC, 128, NHC, 128).transpose(0, 3, 1, 2, 4)
    ).astype(np.float16)
    return wi, wh


def make_in_maps(x, W_i, W_h):
    wi, wh = prep_weights(np.asarray(W_i, np.float32), np.asarray(W_h, np.float32))
    x = np.asarray(x, np.float32)
    # per-core [T, C, LP, BL], batch innermost, zero pad rows at 0 and L+1
    xp = np.zeros((NCORES, T, C, LP, BL), np.float16)
    xc = x.reshape(NCORES, BL, T, C, L).transpose(0, 2, 3, 4, 1)
    xp[:, :, :, 1 : L + 1, :] = xc
    return [
        {"x": np.ascontiguousarray(xp[c]), "wi": wi, "wh": wh}
        for c in range(NCORES)
    ]


def kernel(x, W_i, W_h):
    from concourse.bass_utils import run_bass_kernel_spmd

    nc = _get(1)
    in_maps = make_in_maps(x, W_i, W_h)
    res = run_bass_kernel_spmd(nc, in_maps, core_ids=list(range(NCORES)))
    # per-core o: [T, H, L, BL] -> [BL, T, H, L]
    out = np.concatenate(
        [res.results[c]["o"].transpose(3, 0, 1, 2) for c in range(NCORES)], axis=0
    )
    return np.ascontiguousarray(out.astype(np.float32))
